# revision 10
# baseline (speedup 1.0000x reference)
"""Max-dilated conv2d kernel for Trainium2 (Bass/Tile), 8-core data parallel.

out[b,oc,oh,ow] = max_{ic,kh,kw} x[b,ic,oh+2*kh, ow+2*kw] * w[oc,ic,kh,kw]

Shapes (hardcoded): x (8,32,68,68) f32, w (32,32,3,3) f32, out (8,32,64,64) f32.
stride=1, dilation=2.

Sharding: batch across the 8 NeuronCores (1 image per core), weights replicated.

Per-core algorithm:
  Partition layout p = icq*32 + oc  (icq in 0..3, oc in 0..31).
  The 32 input channels are processed in 8 groups of 4 (ic = h*4 + icq).
  For each group h, x[ic] rows are broadcast 32x across partitions so that
  partition p holds x[h*4 + p//32].  For each (kh,kw) a single fused
  VectorE/GpSimd op does   acc[p] = max(acc[p], x_shifted[p] * wv[p])
  where wv[p] = w[p%32, h*4 + p//32, kh, kw] is a per-partition scalar.
  After all 72 planes, a 2-step cross-partition tree-max (128->64->32)
  reduces the 4 icq slots, leaving out[oc] on partitions 0..31.
"""

import sys

sys.path.insert(0, "/opt/trn_rl_repo")

import numpy as np

import concourse.bacc as bacc
import concourse.tile as tile
from concourse import mybir
from concourse import bass_utils

IC, OC, K = 32, 32, 3
H = W = 68
OH = OW = 64
DH = DW = 2
NCORES = 8
NGROUPS = 8  # ic groups of 4
PLANES = NGROUPS * K * K  # 72
F32 = mybir.dt.float32

# Number of (kh,kw) planes per ic-group handed to GpSimd (0..9); rest on DVE.
N_GPS_PER_GROUP = 3
# 'stt': gpsimd runs the fused scalar_tensor_tensor; 'actmax': ScalarE does the
# multiply and gpsimd only the tensor_max accumulate.
GPS_MODE = "stt"

_cache: dict = {}


def _build(n_gps: int, gps_mode: str = GPS_MODE):
    """Build + compile the per-core Bass program (same NEFF on all cores)."""
    if (n_gps, gps_mode) in _cache:
        return _cache[(n_gps, gps_mode)]

    nc = bacc.Bacc("TRN2", debug=False, num_devices=NCORES)
    x_d = nc.dram_tensor("x", [IC, H, W], F32, kind="ExternalInput").ap()
    wv_d = nc.dram_tensor("wv", [128, PLANES], F32, kind="ExternalInput").ap()
    out_d = nc.dram_tensor("out", [OC, OH, OW], F32, kind="ExternalOutput").ap()

    mult = mybir.AluOpType.mult
    amax = mybir.AluOpType.max

    with tile.TileContext(nc) as tc:
        with (
            tc.tile_pool(name="const", bufs=1) as cpool,
            tc.tile_pool(name="xrep", bufs=4) as xpool,
            tc.tile_pool(name="work", bufs=1) as wpool,
        ):
            wv_sb = cpool.tile([128, PLANES], F32, tag="wv")
            nc.sync.dma_start(wv_sb[:, :], wv_d[:, :])

            acc_v = wpool.tile([128, OH, OW], F32, tag="acc_v")
            if n_gps:
                acc_g = wpool.tile([128, OH, OW], F32, tag="acc_g", name="acc_g")
            else:
                acc_g = None

            first_v = True
            first_g = True
            for h in range(NGROUPS):
                xr = xpool.tile([128, H, W], F32, tag="xr")
                for icq in range(4):
                    src = x_d[h * 4 + icq].unsqueeze(0).broadcast_to([32, H, W])
                    # split each 591KB broadcast across DMA queues
                    for s in range(4):
                        r0, r1 = s * 17, (s + 1) * 17
                        nc.sync.dma_start(
                            xr[icq * 32 : (icq + 1) * 32, r0:r1], src[:, r0:r1]
                        )
                for k in range(K * K):
                    kh, kw = divmod(k, K)
                    j = h * (K * K) + k
                    view = xr[:, DH * kh : DH * kh + OH, DW * kw : DW * kw + OW]
                    wcol = wv_sb[:, j : j + 1]
                    on_gps = k >= (K * K - n_gps)
                    acc = acc_g if on_gps else acc_v
                    if on_gps and gps_mode == "actmax":
                        if first_g:
                            nc.scalar.mul(acc[:], view, wcol)
                            first_g = False
                        else:
                            prod = xpool.tile(
                                [128, OH, OW], F32, tag="prod", name="prod", bufs=3
                            )
                            nc.scalar.mul(prod[:], view, wcol)
                            nc.gpsimd.tensor_max(acc[:], acc[:], prod[:])
                        continue
                    eng = nc.gpsimd if on_gps else nc.vector
                    if (first_g if on_gps else first_v):
                        eng.tensor_scalar_mul(acc[:], view, wcol)
                        if on_gps:
                            first_g = False
                        else:
                            first_v = False
                    else:
                        eng.scalar_tensor_tensor(acc[:], view, wcol, acc[:], mult, amax)

            if n_gps:
                nc.vector.tensor_max(acc_v[:], acc_v[:], acc_g[:])
            # Cross-partition tree-max. TensorTensor requires equal base
            # partitions for SBUF operands, so realign the upper half with
            # SBUF->SBUF DMAs (split across queues) before each max level.
            t64 = wpool.tile([64, OH, OW], F32, tag="t64")
            for s in range(4):
                nc.sync.dma_start(
                    t64[:, s * 16 : (s + 1) * 16, :],
                    acc_v[64:128, s * 16 : (s + 1) * 16, :],
                )
            nc.vector.tensor_max(t64[:], t64[:], acc_v[0:64])
            out_sb = wpool.tile([32, OH, OW], F32, tag="out_sb")
            for s in range(2):
                nc.sync.dma_start(
                    out_sb[:, s * 32 : (s + 1) * 32, :],
                    t64[32:64, s * 32 : (s + 1) * 32, :],
                )
            nc.vector.tensor_max(out_sb[:], out_sb[:], t64[0:32])
            nc.sync.dma_start(out_d[:, :, :], out_sb[:])

    nc.compile()
    _cache[n_gps] = nc
    return nc


def _make_wv(w: np.ndarray) -> np.ndarray:
    """wv[p, h*9+k] = w[p%32, h*4 + p//32, kh, kw] with k = kh*3+kw."""
    wr = w.reshape(OC, NGROUPS, 4, K * K)  # (oc, h, icq, k); ic = h*4+icq
    wv = wr.transpose(2, 0, 1, 3).reshape(4 * OC, PLANES)  # (icq*32+oc, h*9+k)
    return np.ascontiguousarray(wv, dtype=np.float32)


def kernel(x, weight, stride_h=1, stride_w=1, dilation_h=2, dilation_w=2):
    x = np.ascontiguousarray(np.asarray(x, dtype=np.float32))
    w = np.ascontiguousarray(np.asarray(weight, dtype=np.float32))
    assert int(stride_h) == 1 and int(stride_w) == 1
    assert int(dilation_h) == DH and int(dilation_w) == DW
    B = x.shape[0]
    assert x.shape == (B, IC, H, W) and w.shape == (OC, IC, K, K)
    assert B == NCORES

    wv = _make_wv(w)
    nc = _build(N_GPS_PER_GROUP)
    in_maps = [{"x": x[b], "wv": wv} for b in range(B)]
    res = bass_utils.run_bass_kernel_spmd(nc, in_maps, core_ids=list(range(B)))
    out = np.stack([r["out"] for r in res.results], axis=0)
    return out.astype(np.float32)


def run_traced(x, weight, n_gps=N_GPS_PER_GROUP, gps_mode=GPS_MODE, **trace_kwargs):
    """Like kernel() but with hardware profiling; returns (out, BassKernelResults)."""
    x = np.ascontiguousarray(np.asarray(x, dtype=np.float32))
    w = np.ascontiguousarray(np.asarray(weight, dtype=np.float32))
    wv = _make_wv(w)
    nc = _build(n_gps, gps_mode)
    in_maps = [{"x": x[b], "wv": wv} for b in range(x.shape[0])]
    res = bass_utils.run_bass_kernel_spmd(
        nc, in_maps, core_ids=list(range(x.shape[0])), trace=True, **trace_kwargs
    )
    out = np.stack([r["out"] for r in res.results], axis=0)
    return out.astype(np.float32), res


# revision 12
# speedup vs baseline: 1.0166x; 1.0166x over previous
"""Max-dilated conv2d kernel for Trainium2 (Bass/Tile), 8-core data parallel.

out[b,oc,oh,ow] = max_{ic,kh,kw} x[b,ic,oh+2*kh, ow+2*kw] * w[oc,ic,kh,kw]

Shapes (hardcoded): x (8,32,68,68) f32, w (32,32,3,3) f32, out (8,32,64,64) f32.
stride=1, dilation=2.

Sharding: batch across the 8 NeuronCores (1 image per core), weights replicated.

Per-core algorithm:
  Partition layout p = icq*32 + oc  (icq in 0..3, oc in 0..31).
  The 32 input channels are processed in 8 groups of 4 (ic = h*4 + icq).
  For each group h, x[ic] rows are broadcast 32x across partitions so that
  partition p holds x[h*4 + p//32].  For each (kh,kw) a single fused
  VectorE/GpSimd op does   acc[p] = max(acc[p], x_shifted[p] * wv[p])
  where wv[p] = w[p%32, h*4 + p//32, kh, kw] is a per-partition scalar.
  After all 72 planes, a 2-step cross-partition tree-max (128->64->32)
  reduces the 4 icq slots, leaving out[oc] on partitions 0..31.
"""

import sys

sys.path.insert(0, "/opt/trn_rl_repo")

import numpy as np

import concourse.bacc as bacc
import concourse.tile as tile
from concourse import mybir
from concourse import bass_utils

IC, OC, K = 32, 32, 3
H = W = 68
OH = OW = 64
DH = DW = 2
NCORES = 8
NGROUPS = 8  # ic groups of 4
PLANES = NGROUPS * K * K  # 72
F32 = mybir.dt.float32

# Number of (kh,kw) planes per ic-group handed to GpSimd (0..9); rest on DVE.
N_GPS_PER_GROUP = 3
# 'stt': gpsimd runs the fused scalar_tensor_tensor; 'actmax': ScalarE does the
# multiply and gpsimd only the tensor_max accumulate.
GPS_MODE = "stt"

_cache: dict = {}


def _build(n_gps: int, gps_mode: str = GPS_MODE):
    """Build + compile the per-core Bass program (same NEFF on all cores)."""
    if (n_gps, gps_mode) in _cache:
        return _cache[(n_gps, gps_mode)]

    nc = bacc.Bacc("TRN2", debug=False, num_devices=NCORES)
    x_d = nc.dram_tensor("x", [IC, H, W], F32, kind="ExternalInput").ap()
    wv_d = nc.dram_tensor("wv", [128, PLANES], F32, kind="ExternalInput").ap()
    out_d = nc.dram_tensor("out", [OC, OH, OW], F32, kind="ExternalOutput").ap()

    mult = mybir.AluOpType.mult
    amax = mybir.AluOpType.max

    with tile.TileContext(nc) as tc:
        with (
            tc.tile_pool(name="const", bufs=1) as cpool,
            tc.tile_pool(name="xrep", bufs=4) as xpool,
            tc.tile_pool(name="work", bufs=1) as wpool,
        ):
            wv_sb = cpool.tile([128, PLANES], F32, tag="wv")
            nc.sync.dma_start(wv_sb[:, :], wv_d[:, :])

            acc_v = wpool.tile([128, OH, OW], F32, tag="acc_v")

            dma_engines = [nc.sync, nc.scalar, nc.gpsimd]
            first_v = True
            ei = 0
            for h in range(NGROUPS):
                xr = xpool.tile([128, H, W], F32, tag="xr")
                for icq in range(4):
                    src = x_d[h * 4 + icq].unsqueeze(0).broadcast_to([32, H, W])
                    # split each 591KB broadcast across engine sequencers so
                    # dispatch and transfer run in parallel
                    for s in range(2):
                        r0, r1 = s * 34, (s + 1) * 34
                        dma_engines[ei % 3].dma_start(
                            xr[icq * 32 : (icq + 1) * 32, r0:r1], src[:, r0:r1]
                        )
                        ei += 1
                last = h == NGROUPS - 1
                # The last group is pixel-split so the reduction tree's DMAs
                # can overlap the remaining compute.
                splits = [(0, 32), (32, 64)] if last else [(0, 64)]
                for a, b in splits:
                    for k in range(K * K):
                        kh, kw = divmod(k, K)
                        j = h * (K * K) + k
                        view = xr[
                            :, DH * kh + a : DH * kh + b, DW * kw : DW * kw + OW
                        ]
                        wcol = wv_sb[:, j : j + 1]
                        accw = acc_v[:, a:b, :]
                        if first_v:
                            nc.vector.tensor_scalar_mul(accw, view, wcol)
                            first_v = False
                        else:
                            nc.vector.scalar_tensor_tensor(
                                accw, view, wcol, accw, mult, amax
                            )

            # Cross-partition tree-max. TensorTensor requires equal base
            # partitions for SBUF operands, so realign the upper half with
            # SBUF->SBUF DMAs before each max level. Done in two pixel halves
            # so half A's DMAs overlap half B's compute.
            t64 = wpool.tile([64, OH, OW], F32, tag="t64")
            out_sb = wpool.tile([32, OH, OW], F32, tag="out_sb")
            for hi, (a, b) in enumerate([(0, 32), (32, 64)]):
                for s in range(2):
                    r0 = a + s * 16
                    r1 = r0 + 16
                    dma_engines[(hi + s) % 3].dma_start(
                        t64[:, r0:r1, :], acc_v[64:128, r0:r1, :]
                    )
                nc.vector.tensor_max(
                    t64[:, a:b, :], t64[:, a:b, :], acc_v[0:64, a:b, :]
                )
                dma_engines[hi % 3].dma_start(
                    out_sb[:, a:b, :], t64[32:64, a:b, :]
                )
                nc.vector.tensor_max(
                    out_sb[:, a:b, :], out_sb[:, a:b, :], t64[0:32, a:b, :]
                )
                for s in range(2):
                    r0 = a + s * 16
                    r1 = r0 + 16
                    dma_engines[(hi + s) % 3].dma_start(
                        out_d[:, r0:r1, :], out_sb[:, r0:r1, :]
                    )

    nc.compile()
    _cache[n_gps] = nc
    return nc


def _make_wv(w: np.ndarray) -> np.ndarray:
    """wv[p, h*9+k] = w[p%32, h*4 + p//32, kh, kw] with k = kh*3+kw."""
    wr = w.reshape(OC, NGROUPS, 4, K * K)  # (oc, h, icq, k); ic = h*4+icq
    wv = wr.transpose(2, 0, 1, 3).reshape(4 * OC, PLANES)  # (icq*32+oc, h*9+k)
    return np.ascontiguousarray(wv, dtype=np.float32)


def kernel(x, weight, stride_h=1, stride_w=1, dilation_h=2, dilation_w=2):
    x = np.ascontiguousarray(np.asarray(x, dtype=np.float32))
    w = np.ascontiguousarray(np.asarray(weight, dtype=np.float32))
    assert int(stride_h) == 1 and int(stride_w) == 1
    assert int(dilation_h) == DH and int(dilation_w) == DW
    B = x.shape[0]
    assert x.shape == (B, IC, H, W) and w.shape == (OC, IC, K, K)
    assert B == NCORES

    wv = _make_wv(w)
    nc = _build(N_GPS_PER_GROUP)
    in_maps = [{"x": x[b], "wv": wv} for b in range(B)]
    res = bass_utils.run_bass_kernel_spmd(nc, in_maps, core_ids=list(range(B)))
    out = np.stack([r["out"] for r in res.results], axis=0)
    return out.astype(np.float32)


def run_traced(x, weight, n_gps=N_GPS_PER_GROUP, gps_mode=GPS_MODE, **trace_kwargs):
    """Like kernel() but with hardware profiling; returns (out, BassKernelResults)."""
    x = np.ascontiguousarray(np.asarray(x, dtype=np.float32))
    w = np.ascontiguousarray(np.asarray(weight, dtype=np.float32))
    wv = _make_wv(w)
    nc = _build(n_gps, gps_mode)
    in_maps = [{"x": x[b], "wv": wv} for b in range(x.shape[0])]
    res = bass_utils.run_bass_kernel_spmd(
        nc, in_maps, core_ids=list(range(x.shape[0])), trace=True, **trace_kwargs
    )
    out = np.stack([r["out"] for r in res.results], axis=0)
    return out.astype(np.float32), res
